# revision 2
# baseline (speedup 1.0000x reference)
"""Trainium2 Bass kernel for nn_BaseAttention (causal MHA, b=2, n=2048, d=1024, 16 heads).

Sharding (8 cores): core c handles batch c//4 and heads 4*(c%4)..4*(c%4)+3.
- W_q/W_k/W_v column-sharded (256 cols/core), W_o row-sharded (256 rows/core).
- Each core computes a partial output [2048, 1024]; host sums the 4 partials
  per batch (row-parallel out-projection) and stacks the 2 batches.

Per-core kernel (all heavy matmuls in fp32r = TF32-class, 1 cyc/row):
  A. stream x row-groups (512 rows), PE-transpose to x^T
  B. Q^T/K^T (projection emitted transposed), V natural with a ones column
     appended per head ([V|1] trick: AV matmul yields ctx^T rows 0..63 and the
     softmax row-sum at psum partition 64 in one pass)
  C. per (head, q-tile): S^T = K_h @ Q_h^T on PE, exp on ACT (scale=1/8,
     fused), causal mask via gpsimd affine_select (exp-then-zero), AV
     accumulation, normalization via DVE reciprocal + gpsimd
     partition_broadcast + DVE multiply
  D. out-projection from ctx^T with the bias added via a rank-1 (ones x b_o)
     matmul accumulated into the same PSUM bank.
"""
import sys, types

sys.path.insert(0, "/opt/trn_rl_repo")


def _install_ntff_shim():
    # antenv.axon_hooks is absent in this image; register the NTFF profile
    # hook via ctypes so run_bass_kernel_spmd(trace=True) works under axon.
    if "antenv.axon_hooks" in sys.modules:
        return
    try:
        sys.path.insert(0, "/root/.axon_site")
        from trn_agent_boot.trn_boot import _ntff_profile_via_ctypes

        hook = _ntff_profile_via_ctypes("/opt/axon/libaxon_pjrt.so")
        mod = types.ModuleType("antenv.axon_hooks")
        mod.get_axon_ntff_profile_hook = lambda: hook
        mod.set_axon_ntff_profile_hook = lambda h: None
        sys.modules["antenv.axon_hooks"] = mod
    except Exception:
        pass


_install_ntff_shim()

import numpy as np
import concourse.bass as bass
import concourse.mybir as mybir
import concourse.tile as tile
from concourse import bacc
from concourse.bass_utils import run_bass_kernel_spmd
from concourse.masks import make_identity
from contextlib import ExitStack

f32 = mybir.dt.float32
f32r = mybir.dt.float32r
EXP = mybir.ActivationFunctionType.Exp

SEQ = 2048          # sequence length
DIN = 1024          # model dim (8 chunks of 128)
QC = 256            # q/k/v cols per core (4 heads x 64)
HD = 64             # head dim
NH = 4              # heads per core
NG = 4              # row groups of 512
VST = NH * 65       # Vones stride per row chunk (4 heads x (64 V + 1 ones))

TRACE = False
LAST_RESULTS = None


def build_nc():
    nc = bacc.Bacc()
    x_d = nc.dram_tensor("x", [SEQ, DIN], f32, kind="ExternalInput")
    wq_d = nc.dram_tensor("wq", [DIN, QC], f32, kind="ExternalInput")
    wk_d = nc.dram_tensor("wk", [DIN, QC], f32, kind="ExternalInput")
    wv_d = nc.dram_tensor("wv", [DIN, QC], f32, kind="ExternalInput")
    wo_d = nc.dram_tensor("wo", [QC, DIN], f32, kind="ExternalInput")
    bo_d = nc.dram_tensor("bo", [1, DIN], f32, kind="ExternalInput")
    out_d = nc.dram_tensor("out", [SEQ, DIN], f32, kind="ExternalOutput")

    with tile.TileContext(nc) as tc, ExitStack() as ctx:
        cst = ctx.enter_context(tc.tile_pool(name="cst", bufs=1))
        wr = ctx.enter_context(tc.tile_pool(name="wr", bufs=1))
        xst = ctx.enter_context(tc.tile_pool(name="xst", bufs=4))
        xt = ctx.enter_context(tc.tile_pool(name="xt", bufs=2))
        big = ctx.enter_context(tc.tile_pool(name="big", bufs=1))
        ptp = ctx.enter_context(tc.tile_pool(name="ptp", bufs=4))
        nrm = ctx.enter_context(tc.tile_pool(name="nrm", bufs=2))
        ob = ctx.enter_context(tc.tile_pool(name="ob", bufs=3))
        ps = ctx.enter_context(tc.tile_pool(name="ps", bufs=1, space="PSUM"))

        # ---- constants ----
        ident = cst.tile([128, 128], f32)
        make_identity(nc, ident[:])

        ones_f = cst.tile([128, 64], f32)
        nc.vector.memset(ones_f[:], 1.0)
        ones_row = cst.tile([1, 128], f32r)
        nc.vector.tensor_copy(ones_row[0:1, 0:64], ones_f[0:1, :])
        nc.vector.tensor_copy(ones_row[0:1, 64:128], ones_f[0:1, :])

        # ---- weights (DMA staging f32 -> round to f32r) ----
        wq_r = wr.tile([128, 8 * QC], f32r)
        wk_r = wr.tile([128, 8 * QC], f32r)
        wv_r = wr.tile([128, 8 * QC], f32r)
        wo_r = wr.tile([128, 2 * DIN], f32r)
        bo_r = cst.tile([1, DIN], f32r)

        for wd, wt in ((wq_d, wq_r), (wk_d, wk_r), (wv_d, wv_r)):
            src = wd.rearrange("(c p) n -> p c n", p=128)  # [128, 8, 256]
            for half in range(2):
                stg = xst.tile([128, 1024], f32, tag="x", name=f"wstg")
                nc.sync.dma_start(
                    stg[:].rearrange("p (c n) -> p c n", n=QC),
                    src[:, half * 4:(half + 1) * 4, :],
                )
                nc.vector.tensor_copy(wt[:, half * 1024:(half + 1) * 1024], stg[:])
        src = wo_d.rearrange("(u p) n -> p u n", p=128)  # [128, 2, 1024]
        for u in range(2):
            stg = xst.tile([128, 1024], f32, tag="x", name="wstg")
            nc.sync.dma_start(stg[:], src[:, u, :])
            nc.vector.tensor_copy(wo_r[:, u * DIN:(u + 1) * DIN], stg[:])
        stg = xst.tile([128, 1024], f32, tag="x", name="wstg")
        nc.sync.dma_start(stg[0:1, :], bo_d[:])
        nc.vector.tensor_copy(bo_r[:], stg[0:1, :])

        # ---- persistent activations ----
        qt_sb = [big.tile([128, SEQ], f32r, name=f"qt{t}") for t in range(2)]
        kt_sb = [big.tile([128, SEQ], f32r, name=f"kt{t}") for t in range(2)]
        vones = big.tile([128, 16 * VST], f32r)
        ctxt = [big.tile([128, SEQ], f32r, name=f"ctxt{t}") for t in range(2)]

        # init the ones columns of vones: view [128, 16, 4, 65] col 64
        vview = vones.rearrange("p (r h e) -> p r h e", h=NH, e=65)
        nc.vector.tensor_copy(
            vview[:, :, :, 64],
            ones_f[:].rearrange("p (r h) -> p r h", h=NH),
        )

        # ---- main loop: per row group g, transpose+project, then attention j=g ----
        for g in range(NG):
            # stage x rows of this group
            x_ts = []
            for rc in range(4):
                rcg = 4 * g + rc
                x_t = xst.tile([128, DIN], f32, tag="x", name=f"x_{rcg}")
                nc.sync.dma_start(x_t[:], x_d[rcg * 128:(rcg + 1) * 128, :])
                x_ts.append(x_t)
            # transpose to xTg [128, 8*512]; xTg[p, c*512 + r] = x[g*512+r, c*128+p]
            xTg = xt.tile([128, 8 * 512], f32r, name="xTg")
            for c in range(8):
                tp = ps.tile([128, 512], f32, tag="a", bufs=3, name="tps")
                for rc in range(4):
                    nc.tensor.transpose(
                        tp[:, rc * 128:(rc + 1) * 128],
                        x_ts[rc][:, c * 128:(c + 1) * 128],
                        ident[:],
                    )
                nc.scalar.copy(xTg[:, c * 512:(c + 1) * 512], tp[:])

            # Q^T / K^T projections for this group's rows
            for t in range(2):
                for which, wt, dst in (("q", wq_r, qt_sb), ("k", wk_r, kt_sb)):
                    prj = ps.tile([128, 512], f32, tag="b", bufs=2, name="prj")
                    for c in range(8):
                        nc.tensor.matmul(
                            prj[:],
                            wt[:, c * QC + t * 128: c * QC + t * 128 + 128],
                            xTg[:, c * 512:(c + 1) * 512],
                            start=(c == 0),
                            stop=(c == 7),
                        )
                    nc.vector.tensor_copy(dst[t][:, g * 512:(g + 1) * 512], prj[:])

            # V projection (natural layout), interleaved into vones
            for rc in range(4):
                rcg = 4 * g + rc
                vps = ps.tile([128, 256], f32, tag="c", bufs=1, name="vps")
                for c in range(8):
                    nc.tensor.matmul(
                        vps[:],
                        xTg[:, c * 512 + rc * 128: c * 512 + rc * 128 + 128],
                        wv_r[:, c * QC:(c + 1) * QC],
                        start=(c == 0),
                        stop=(c == 7),
                    )
                nc.vector.tensor_copy(
                    vview[:, rcg, :, 0:64],
                    vps[:].rearrange("p (h e) -> p h e", e=HD),
                )

            # ---- attention for q-tile j = g ----
            j = g
            imax = 4 * j + 3
            for h in range(NH):
                u, o = h // 2, (h % 2) * 64
                av = ps.tile([65, 512], f32, tag="av", bufs=2, name="av")
                pts = []
                for i in range(imax + 1):
                    sps = ps.tile([128, 512], f32, tag="a", bufs=3, name="sps")
                    nc.tensor.matmul(
                        sps[:],
                        kt_sb[u][o:o + 64, i * 128:(i + 1) * 128],
                        qt_sb[u][o:o + 64, j * 512:(j + 1) * 512],
                        start=True,
                        stop=True,
                    )
                    pt = ptp.tile([128, 512], f32r, tag="pt", name="pt")
                    nc.scalar.activation(pt[:], sps[:], EXP, scale=0.125)
                    if i >= 4 * j:
                        nc.gpsimd.affine_select(
                            out=pt[:],
                            in_=pt[:],
                            compare_op=mybir.AluOpType.is_ge,
                            fill=0.0,
                            base=512 * j - 128 * i,
                            channel_multiplier=-1,
                            pattern=[[1, 512]],
                        )
                    pts.append(pt)
                    # software-pipeline AV one step behind S
                    if i >= 1:
                        k = i - 1
                        nc.tensor.matmul(
                            av[:],
                            vones[:, k * VST + h * 65: k * VST + h * 65 + 65],
                            pts[k][:],
                            start=(k == 0),
                            stop=False,
                        )
                nc.tensor.matmul(
                    av[:],
                    vones[:, imax * VST + h * 65: imax * VST + h * 65 + 65],
                    pts[imax][:],
                    start=(imax == 0),
                    stop=True,
                )
                # normalize: ctx^T = avc[0:64] * (1/rowsum) broadcast
                rinv = nrm.tile([1, 512], f32, tag="rinv", name="rinv")
                nc.vector.reciprocal(rinv[:], av[64:65, :])
                bcast = nrm.tile([64, 512], f32, tag="bcast", name="bcast")
                nc.gpsimd.partition_broadcast(bcast[:], rinv[:])
                nc.vector.tensor_mul(
                    ctxt[u][o:o + 64, j * 512:(j + 1) * 512],
                    av[0:64, :],
                    bcast[:],
                )

        # ---- out projection ----
        for rc in range(16):
            for n in range(2):
                ops = ps.tile([128, 512], f32, tag="a", bufs=3, name="ops")
                for u in range(2):
                    nc.tensor.matmul(
                        ops[:],
                        ctxt[u][:, rc * 128:(rc + 1) * 128],
                        wo_r[:, u * DIN + n * 512: u * DIN + n * 512 + 512],
                        start=(u == 0),
                        stop=False,
                    )
                nc.tensor.matmul(
                    ops[:],
                    ones_row[:],
                    bo_r[0:1, n * 512:(n + 1) * 512],
                    start=False,
                    stop=True,
                )
                osb = ob.tile([128, 512], f32, tag="o", name="osb")
                nc.vector.tensor_copy(osb[:], ops[:])
                nc.sync.dma_start(
                    out_d[rc * 128:(rc + 1) * 128, n * 512:(n + 1) * 512], osb[:]
                )

    nc.compile()
    return nc


_NC = None


def _get_nc():
    global _NC
    if _NC is None:
        _NC = build_nc()
    return _NC


def kernel(x, W_q, W_k, W_v, W_o, b_o):
    global LAST_RESULTS
    nc = _get_nc()
    x = np.ascontiguousarray(np.asarray(x, np.float32))
    W_q = np.asarray(W_q, np.float32)
    W_k = np.asarray(W_k, np.float32)
    W_v = np.asarray(W_v, np.float32)
    W_o = np.asarray(W_o, np.float32)
    b_o = np.asarray(b_o, np.float32)
    zeros_bo = np.zeros((1, DIN), np.float32)

    in_maps = []
    for c in range(8):
        bi, g = c // 4, c % 4
        sl = slice(g * QC, (g + 1) * QC)
        in_maps.append({
            "x": x[bi],
            "wq": np.ascontiguousarray(W_q[:, sl]),
            "wk": np.ascontiguousarray(W_k[:, sl]),
            "wv": np.ascontiguousarray(W_v[:, sl]),
            "wo": np.ascontiguousarray(W_o[sl, :]),
            "bo": b_o.reshape(1, DIN) if g == 0 else zeros_bo,
        })

    res = run_bass_kernel_spmd(nc, in_maps, list(range(8)), trace=TRACE)
    LAST_RESULTS = res
    outs = [r["out"] for r in res.results]
    return np.stack([
        outs[0] + outs[1] + outs[2] + outs[3],
        outs[4] + outs[5] + outs[6] + outs[7],
    ])


if __name__ == "__main__":
    if "--compile-only" in sys.argv:
        import tempfile
        from concourse.bass_utils import compile_bass_kernel

        nc = build_nc()
        with tempfile.TemporaryDirectory() as td:
            print("walrus compiling...")
            neff = compile_bass_kernel(nc, td)
            print("COMPILE OK", neff)


# revision 4
# speedup vs baseline: 1.0292x; 1.0292x over previous
"""Trainium2 Bass kernel for nn_BaseAttention (causal MHA, b=2, n=2048, d=1024, 16 heads).

Sharding (8 cores): core c handles batch c//4 and heads 4*(c%4)..4*(c%4)+3.
- W_q/W_k/W_v column-sharded (256 cols/core), W_o row-sharded (256 rows/core).
- Each core computes a partial output [2048, 1024] in fp32; host sums the 4
  partials per batch (row-parallel out-projection) and stacks the 2 batches.

Per-core kernel (bf16 data path, fp32 PSUM accumulation):
  A. x (converted to bf16 on host) is DMA-transposed straight into SBUF as
     x^T; no PE transposes needed.
  B. Q^T/K^T (projection emitted transposed), V natural with a ones column
     appended per head ([V|1] trick: the AV matmul yields ctx^T on psum
     partitions 0..63 and the softmax row-sum at partition 64 in one pass).
  C. per (head, q-tile): S^T = K_h @ Q_h^T on PE, exp on ACT (scale=1/8
     fused), causal mask via gpsimd affine_select (exp-then-zero), AV
     accumulation, normalization via DVE reciprocal + gpsimd
     partition_broadcast + DVE multiply.
  D. out-projection from ctx^T with the bias added via a rank-1 (ones x b_o)
     matmul accumulated into the same PSUM bank.
"""
import sys, types

sys.path.insert(0, "/opt/trn_rl_repo")


def _install_ntff_shim():
    # antenv.axon_hooks is absent in this image; register the NTFF profile
    # hook via ctypes so run_bass_kernel_spmd(trace=True) works under axon.
    if "antenv.axon_hooks" in sys.modules:
        return
    try:
        sys.path.insert(0, "/root/.axon_site")
        from trn_agent_boot.trn_boot import _ntff_profile_via_ctypes

        hook = _ntff_profile_via_ctypes("/opt/axon/libaxon_pjrt.so")
        mod = types.ModuleType("antenv.axon_hooks")
        mod.get_axon_ntff_profile_hook = lambda: hook
        mod.set_axon_ntff_profile_hook = lambda h: None
        sys.modules["antenv.axon_hooks"] = mod
    except Exception:
        pass


_install_ntff_shim()

import numpy as np
import ml_dtypes
import concourse.bass as bass
import concourse.mybir as mybir
import concourse.tile as tile
from concourse import bacc
from concourse.bass_utils import run_bass_kernel_spmd
from contextlib import ExitStack

f32 = mybir.dt.float32
bf16 = mybir.dt.bfloat16
EXP = mybir.ActivationFunctionType.Exp

SEQ = 2048          # sequence length
DIN = 1024          # model dim (8 chunks of 128)
QC = 256            # q/k/v cols per core (4 heads x 64)
HD = 64             # head dim
NH = 4              # heads per core
NG = 4              # row groups of 512
VST = NH * 65       # Vones stride per row chunk (4 heads x (64 V + 1 ones))

TRACE = False
LAST_RESULTS = None


def build_nc():
    nc = bacc.Bacc()
    x_d = nc.dram_tensor("x", [SEQ, DIN], bf16, kind="ExternalInput")
    wq_d = nc.dram_tensor("wq", [DIN, QC], bf16, kind="ExternalInput")
    wk_d = nc.dram_tensor("wk", [DIN, QC], bf16, kind="ExternalInput")
    wv_d = nc.dram_tensor("wv", [DIN, QC], bf16, kind="ExternalInput")
    wo_d = nc.dram_tensor("wo", [QC, DIN], bf16, kind="ExternalInput")
    bo_d = nc.dram_tensor("bo", [1, DIN], bf16, kind="ExternalInput")
    out_d = nc.dram_tensor("out", [SEQ, DIN], f32, kind="ExternalOutput")

    with tile.TileContext(nc) as tc, ExitStack() as ctx:
        cst = ctx.enter_context(tc.tile_pool(name="cst", bufs=1))
        wr = ctx.enter_context(tc.tile_pool(name="wr", bufs=1))
        big = ctx.enter_context(tc.tile_pool(name="big", bufs=1))
        ptp = ctx.enter_context(tc.tile_pool(name="ptp", bufs=6))
        nrm = ctx.enter_context(tc.tile_pool(name="nrm", bufs=2))
        ob = ctx.enter_context(tc.tile_pool(name="ob", bufs=3))
        ps = ctx.enter_context(tc.tile_pool(name="ps", bufs=1, space="PSUM"))

        # ---- constants ----
        ones_row = cst.tile([1, 128], bf16)
        nc.vector.memset(ones_row[:], 1.0)

        # ---- weights: DMA directly (already bf16 in DRAM) ----
        # wq_sb[p, c*256+n] = W_q[c*128+p, n]
        wq_sb = wr.tile([128, 8 * QC], bf16)
        nc.sync.dma_start(
            wq_sb[:].rearrange("p (c n) -> p c n", n=QC),
            wq_d.rearrange("(c p) n -> p c n", p=128),
        )
        wk_sb = wr.tile([128, 8 * QC], bf16)
        nc.sync.dma_start(
            wk_sb[:].rearrange("p (c n) -> p c n", n=QC),
            wk_d.rearrange("(c p) n -> p c n", p=128),
        )
        wv_sb = wr.tile([128, 8 * QC], bf16)
        nc.sync.dma_start(
            wv_sb[:].rearrange("p (c n) -> p c n", n=QC),
            wv_d.rearrange("(c p) n -> p c n", p=128),
        )
        # wo_sb[p, u*1024+n] = W_o[u*128+p, n]
        wo_sb = wr.tile([128, 2 * DIN], bf16)
        nc.sync.dma_start(
            wo_sb[:].rearrange("p (u n) -> p u n", n=DIN),
            wo_d.rearrange("(u p) n -> p u n", p=128),
        )
        bo_sb = cst.tile([1, DIN], bf16)
        nc.sync.dma_start(bo_sb[:], bo_d[:])

        # ---- x^T via DMA transpose: xT[p, c*2048+r] = x[r, c*128+p] ----
        xT = big.tile([128, 8 * SEQ], bf16)
        for c in range(8):
            nc.sync.dma_start_transpose(
                xT[:, c * SEQ:(c + 1) * SEQ],
                x_d[:, c * 128:(c + 1) * 128],
            )

        # ---- persistent activations ----
        qt_sb = [big.tile([128, SEQ], bf16, name=f"qt{t}") for t in range(2)]
        kt_sb = [big.tile([128, SEQ], bf16, name=f"kt{t}") for t in range(2)]
        vones = big.tile([128, 16 * VST], bf16)
        ctxt = [big.tile([128, SEQ], bf16, name=f"ctxt{t}") for t in range(2)]

        # ones columns of vones: view [128, 16, 4, 65] col 64
        vview = vones.rearrange("p (r h e) -> p r h e", h=NH, e=65)
        nc.vector.memset(vview[:, :, :, 64], 1.0)

        # ---- main loop: per row group g project, then attention j=g ----
        for g in range(NG):
            # Q^T / K^T projections for this group's rows
            for t in range(2):
                for wt, dst in ((wq_sb, qt_sb), (wk_sb, kt_sb)):
                    prj = ps.tile([128, 512], f32, tag="b", bufs=2, name="prj")
                    for c in range(8):
                        nc.tensor.matmul(
                            prj[:],
                            wt[:, c * QC + t * 128: c * QC + t * 128 + 128],
                            xT[:, c * SEQ + g * 512: c * SEQ + g * 512 + 512],
                            start=(c == 0),
                            stop=(c == 7),
                        )
                    nc.vector.tensor_copy(dst[t][:, g * 512:(g + 1) * 512], prj[:])

            # V projection (natural layout), interleaved into vones
            for rc in range(4):
                rcg = 4 * g + rc
                vps = ps.tile([128, 256], f32, tag="c", bufs=1, name="vps")
                for c in range(8):
                    nc.tensor.matmul(
                        vps[:],
                        xT[:, c * SEQ + rcg * 128: c * SEQ + rcg * 128 + 128],
                        wv_sb[:, c * QC:(c + 1) * QC],
                        start=(c == 0),
                        stop=(c == 7),
                    )
                nc.vector.tensor_copy(
                    vview[:, rcg, :, 0:64],
                    vps[:].rearrange("p (h e) -> p h e", e=HD),
                )

            # ---- attention for q-tile j = g ----
            j = g
            imax = 4 * j + 3
            for h in range(NH):
                u, o = h // 2, (h % 2) * 64
                av = ps.tile([65, 512], f32, tag="av", bufs=2, name="av")
                pts = []
                for i in range(imax + 1):
                    sps = ps.tile([128, 512], f32, tag="a", bufs=3, name="sps")
                    nc.tensor.matmul(
                        sps[:],
                        kt_sb[u][o:o + 64, i * 128:(i + 1) * 128],
                        qt_sb[u][o:o + 64, j * 512:(j + 1) * 512],
                        start=True,
                        stop=True,
                    )
                    pt = ptp.tile([128, 512], bf16, tag="pt", name="pt")
                    nc.scalar.activation(pt[:], sps[:], EXP, scale=0.125)
                    if i >= 4 * j:
                        nc.gpsimd.affine_select(
                            out=pt[:],
                            in_=pt[:],
                            compare_op=mybir.AluOpType.is_ge,
                            fill=0.0,
                            base=512 * j - 128 * i,
                            channel_multiplier=-1,
                            pattern=[[1, 512]],
                        )
                    pts.append(pt)
                    # software-pipeline AV one step behind S
                    if i >= 1:
                        k = i - 1
                        nc.tensor.matmul(
                            av[:],
                            vones[:, k * VST + h * 65: k * VST + h * 65 + 65],
                            pts[k][:],
                            start=(k == 0),
                            stop=False,
                        )
                nc.tensor.matmul(
                    av[:],
                    vones[:, imax * VST + h * 65: imax * VST + h * 65 + 65],
                    pts[imax][:],
                    start=(imax == 0),
                    stop=True,
                )
                # normalize: ctx^T = av[0:64] * (1/rowsum) broadcast
                rinv = nrm.tile([1, 512], f32, tag="rinv", name="rinv")
                nc.vector.reciprocal(rinv[:], av[64:65, :])
                bcast = nrm.tile([64, 512], f32, tag="bcast", name="bcast")
                nc.gpsimd.partition_broadcast(bcast[:], rinv[:])
                nc.vector.tensor_mul(
                    ctxt[u][o:o + 64, j * 512:(j + 1) * 512],
                    av[0:64, :],
                    bcast[:],
                )

        # ---- out projection ----
        for rc in range(16):
            for n in range(2):
                ops = ps.tile([128, 512], f32, tag="a", bufs=3, name="ops")
                for u in range(2):
                    nc.tensor.matmul(
                        ops[:],
                        ctxt[u][:, rc * 128:(rc + 1) * 128],
                        wo_sb[:, u * DIN + n * 512: u * DIN + n * 512 + 512],
                        start=(u == 0),
                        stop=False,
                    )
                nc.tensor.matmul(
                    ops[:],
                    ones_row[:],
                    bo_sb[0:1, n * 512:(n + 1) * 512],
                    start=False,
                    stop=True,
                )
                osb = ob.tile([128, 512], f32, tag="o", name="osb")
                nc.vector.tensor_copy(osb[:], ops[:])
                nc.sync.dma_start(
                    out_d[rc * 128:(rc + 1) * 128, n * 512:(n + 1) * 512], osb[:]
                )

    nc.compile()
    return nc


_NC = None


def _get_nc():
    global _NC
    if _NC is None:
        _NC = build_nc()
    return _NC


def kernel(x, W_q, W_k, W_v, W_o, b_o):
    global LAST_RESULTS
    nc = _get_nc()
    bf = ml_dtypes.bfloat16
    x = np.ascontiguousarray(np.asarray(x, np.float32)).astype(bf)
    W_q = np.asarray(W_q, np.float32).astype(bf)
    W_k = np.asarray(W_k, np.float32).astype(bf)
    W_v = np.asarray(W_v, np.float32).astype(bf)
    W_o = np.asarray(W_o, np.float32).astype(bf)
    b_o = np.asarray(b_o, np.float32).astype(bf).reshape(1, DIN)
    zeros_bo = np.zeros((1, DIN), bf)

    in_maps = []
    for c in range(8):
        bi, g = c // 4, c % 4
        sl = slice(g * QC, (g + 1) * QC)
        in_maps.append({
            "x": x[bi],
            "wq": np.ascontiguousarray(W_q[:, sl]),
            "wk": np.ascontiguousarray(W_k[:, sl]),
            "wv": np.ascontiguousarray(W_v[:, sl]),
            "wo": np.ascontiguousarray(W_o[sl, :]),
            "bo": b_o if g == 0 else zeros_bo,
        })

    res = run_bass_kernel_spmd(nc, in_maps, list(range(8)), trace=TRACE)
    LAST_RESULTS = res
    outs = [r["out"] for r in res.results]
    return np.stack([
        outs[0] + outs[1] + outs[2] + outs[3],
        outs[4] + outs[5] + outs[6] + outs[7],
    ])


if __name__ == "__main__":
    if "--compile-only" in sys.argv:
        import tempfile
        from concourse.bass_utils import compile_bass_kernel

        nc = build_nc()
        with tempfile.TemporaryDirectory() as td:
            print("walrus compiling...")
            neff = compile_bass_kernel(nc, td)
            print("COMPILE OK", neff)


# revision 9
# speedup vs baseline: 1.3177x; 1.2803x over previous
"""Trainium2 Bass kernel for nn_BaseAttention (causal MHA, b=2, n=2048, d=1024, 16 heads).

Sharding (8 cores): core c handles batch c//4 and heads 4*(c%4)..4*(c%4)+3.
- W_q/W_k/W_v column-sharded (256 cols/core), W_o row-sharded (256 rows/core).
- Each core computes a partial output [2048, 1024] in fp32; host sums the 4
  partials per batch (row-parallel out-projection) and stacks the 2 batches.

Per-core kernel (bf16 data path, fp32 PSUM accumulation):
  A. x (converted to bf16 on host) is DMA-transposed straight into SBUF as
     x^T; no PE transposes needed.
  B. Q^T/K^T (projection emitted transposed), V natural with a ones column
     appended per head ([V|1] trick: the AV matmul yields ctx^T on psum
     partitions 0..63 and the softmax row-sum at partition 64 in one pass).
  C. per (head, q-tile): S^T = K_h @ Q_h^T on PE, exp on ACT (scale=1/8
     fused), causal mask via gpsimd affine_select (exp-then-zero), AV
     accumulation, normalization via DVE reciprocal + gpsimd
     partition_broadcast + DVE multiply.
  D. out-projection from ctx^T with the bias added via a rank-1 (ones x b_o)
     matmul accumulated into the same PSUM bank.
"""
import sys, types

sys.path.insert(0, "/opt/trn_rl_repo")


def _install_ntff_shim():
    # antenv.axon_hooks is absent in this image; register the NTFF profile
    # hook via ctypes so run_bass_kernel_spmd(trace=True) works under axon.
    if "antenv.axon_hooks" in sys.modules:
        return
    try:
        sys.path.insert(0, "/root/.axon_site")
        from trn_agent_boot.trn_boot import _ntff_profile_via_ctypes

        hook = _ntff_profile_via_ctypes("/opt/axon/libaxon_pjrt.so")
        mod = types.ModuleType("antenv.axon_hooks")
        mod.get_axon_ntff_profile_hook = lambda: hook
        mod.set_axon_ntff_profile_hook = lambda h: None
        sys.modules["antenv.axon_hooks"] = mod
    except Exception:
        pass


_install_ntff_shim()

import numpy as np
import ml_dtypes
import concourse.bass as bass
import concourse.mybir as mybir
import concourse.tile as tile
from concourse import bacc
from concourse.bass_utils import run_bass_kernel_spmd
from contextlib import ExitStack

f32 = mybir.dt.float32
bf16 = mybir.dt.bfloat16
EXP = mybir.ActivationFunctionType.Exp

SEQ = 2048          # sequence length
DIN = 1024          # model dim (8 chunks of 128)
QC = 256            # q/k/v cols per core (4 heads x 64)
HD = 64             # head dim
NH = 4              # heads per core
NG = 4              # row groups of 512
VST = NH * 65       # Vones stride per row chunk (4 heads x (64 V + 1 ones))

TRACE = False
LAST_RESULTS = None


def build_nc():
    nc = bacc.Bacc()
    x_d = nc.dram_tensor("x", [SEQ, DIN], bf16, kind="ExternalInput")
    wq_d = nc.dram_tensor("wq", [DIN, QC], bf16, kind="ExternalInput")
    wk_d = nc.dram_tensor("wk", [DIN, QC], bf16, kind="ExternalInput")
    wv_d = nc.dram_tensor("wv", [DIN, QC], bf16, kind="ExternalInput")
    wo_d = nc.dram_tensor("wo", [QC, DIN], bf16, kind="ExternalInput")
    bo_d = nc.dram_tensor("bo", [1, DIN], bf16, kind="ExternalInput")
    out_d = nc.dram_tensor("out", [SEQ, DIN], f32, kind="ExternalOutput")

    with tile.TileContext(nc) as tc, ExitStack() as ctx:
        cst = ctx.enter_context(tc.tile_pool(name="cst", bufs=1))
        wr = ctx.enter_context(tc.tile_pool(name="wr", bufs=1))
        big = ctx.enter_context(tc.tile_pool(name="big", bufs=1))
        ptp = ctx.enter_context(tc.tile_pool(name="ptp", bufs=6))
        nrm = ctx.enter_context(tc.tile_pool(name="nrm", bufs=2))
        ob = ctx.enter_context(tc.tile_pool(name="ob", bufs=3))
        ps = ctx.enter_context(tc.tile_pool(name="ps", bufs=1, space="PSUM"))

        # ---- constants ----
        ones_row = cst.tile([1, 128], bf16)
        nc.vector.memset(ones_row[:], 1.0)

        # ---- weights: DMA directly (already bf16 in DRAM) ----
        # wq_sb[p, c*256+n] = W_q[c*128+p, n]
        wq_sb = wr.tile([128, 8 * QC], bf16)
        nc.sync.dma_start(
            wq_sb[:].rearrange("p (c n) -> p c n", n=QC),
            wq_d.rearrange("(c p) n -> p c n", p=128),
        )
        wk_sb = wr.tile([128, 8 * QC], bf16)
        nc.sync.dma_start(
            wk_sb[:].rearrange("p (c n) -> p c n", n=QC),
            wk_d.rearrange("(c p) n -> p c n", p=128),
        )
        wv_sb = wr.tile([128, 8 * QC], bf16)
        nc.sync.dma_start(
            wv_sb[:].rearrange("p (c n) -> p c n", n=QC),
            wv_d.rearrange("(c p) n -> p c n", p=128),
        )
        # wo_sb[p, u*1024+n] = W_o[u*128+p, n]
        wo_sb = wr.tile([128, 2 * DIN], bf16)
        nc.sync.dma_start(
            wo_sb[:].rearrange("p (u n) -> p u n", n=DIN),
            wo_d.rearrange("(u p) n -> p u n", p=128),
        )
        bo_sb = cst.tile([1, DIN], bf16)
        nc.sync.dma_start(bo_sb[:], bo_d[:])

        # ---- x^T via DMA transpose: xT[p, c*2048+r] = x[r, c*128+p] ----
        # split by row group so group-0 projections can start early
        xT = big.tile([128, 8 * SEQ], bf16)
        for g in range(NG):
            for c in range(8):
                nc.sync.dma_start_transpose(
                    xT[:, c * SEQ + g * 512: c * SEQ + (g + 1) * 512],
                    x_d[g * 512:(g + 1) * 512, c * 128:(c + 1) * 128],
                )

        # ---- persistent activations ----
        qt_sb = [big.tile([128, SEQ], bf16, name=f"qt{t}") for t in range(2)]
        kt_sb = [big.tile([128, SEQ], bf16, name=f"kt{t}") for t in range(2)]
        vones = big.tile([128, 16 * VST], bf16)
        ctxt = [big.tile([128, SEQ], bf16, name=f"ctxt{t}") for t in range(2)]

        # ones columns of vones: view [128, 16, 4, 65] col 64
        vview = vones.rearrange("p (r h e) -> p r h e", h=NH, e=65)
        nc.vector.memset(vview[:, :, :, 64], 1.0)

        # ---- main loop: per row group g project, then attention j=g ----
        for g in range(NG):
            # Q^T / K^T projections for this group's rows
            for t in range(2):
                for wt, dst in ((wq_sb, qt_sb), (wk_sb, kt_sb)):
                    prj = ps.tile([128, 512], f32, tag="b", bufs=2, name="prj")
                    for c in range(8):
                        nc.tensor.matmul(
                            prj[:],
                            wt[:, c * QC + t * 128: c * QC + t * 128 + 128],
                            xT[:, c * SEQ + g * 512: c * SEQ + g * 512 + 512],
                            start=(c == 0),
                            stop=(c == 7),
                        )
                    nc.vector.tensor_copy(dst[t][:, g * 512:(g + 1) * 512], prj[:])

            # V projection (natural layout), interleaved into vones
            for rc in range(4):
                rcg = 4 * g + rc
                vps = ps.tile([128, 256], f32, tag="c", bufs=1, name="vps")
                for c in range(8):
                    nc.tensor.matmul(
                        vps[:],
                        xT[:, c * SEQ + rcg * 128: c * SEQ + rcg * 128 + 128],
                        wv_sb[:, c * QC:(c + 1) * QC],
                        start=(c == 0),
                        stop=(c == 7),
                    )
                nc.vector.tensor_copy(
                    vview[:, rcg, :, 0:64],
                    vps[:].rearrange("p (h e) -> p h e", e=HD),
                )

            # ---- attention for q-tile j = g ----
            # heads processed in pairs (even head on PE rows 0-63, odd head on
            # rows 64-127 -> concurrent S matmuls via row tiling); AV pipelined
            # one step behind S; normalization deferred (batched reciprocal).
            j = g
            imax = 4 * j + 3
            for u in range(2):           # head pair u: heads 2u, 2u+1
                avs = [ps.tile([65, 512], f32, tag="av", bufs=2, name=f"av{p}")
                       for p in range(2)]
                pts = [[], []]
                for i in range(imax + 1):
                    cur = []
                    for p in range(2):   # p = head parity
                        h, o = 2 * u + p, p * 64
                        sps = ps.tile([128, 512], f32, tag="a", bufs=3, name="sps")
                        nc.tensor.matmul(
                            sps[:],
                            kt_sb[u][o:o + 64, i * 128:(i + 1) * 128],
                            qt_sb[u][o:o + 64, j * 512:(j + 1) * 512],
                            start=True,
                            stop=True,
                        )
                        cur.append(sps)
                    for p in range(2):
                        h = 2 * u + p
                        pt = ptp.tile([128, 512], bf16, tag="pt", name="pt")
                        nc.scalar.activation(pt[:], cur[p][:], EXP, scale=0.125)
                        if i >= 4 * j:
                            nc.gpsimd.affine_select(
                                out=pt[:],
                                in_=pt[:],
                                compare_op=mybir.AluOpType.is_ge,
                                fill=0.0,
                                base=512 * j - 128 * i,
                                channel_multiplier=-1,
                                pattern=[[1, 512]],
                            )
                        pts[p].append(pt)
                    if i >= 1:
                        k = i - 1
                        for p in range(2):
                            h = 2 * u + p
                            nc.tensor.matmul(
                                avs[p][:],
                                vones[:, k * VST + h * 65: k * VST + h * 65 + 65],
                                pts[p][k][:],
                                start=(k == 0),
                                stop=False,
                            )
                for p in range(2):
                    h = 2 * u + p
                    nc.tensor.matmul(
                        avs[p][:],
                        vones[:, imax * VST + h * 65: imax * VST + h * 65 + 65],
                        pts[p][imax][:],
                        start=(imax == 0),
                        stop=True,
                    )
                # normalize: ctx^T = av[0:64] * (1/rowsum) broadcast
                for p in range(2):
                    o = p * 64
                    rsrow = nrm.tile([1, 512], f32, tag="rsrow", name="rsrow")
                    nc.vector.tensor_copy(rsrow[:], avs[p][64:65, :])
                    rinv = nrm.tile([1, 512], f32, tag="rinv", name="rinv")
                    nc.vector.reciprocal_approx_fast(rinv[:], rsrow[:])
                    bcast = nrm.tile([64, 512], f32, tag="bcast", name="bcast")
                    nc.gpsimd.partition_broadcast(bcast[:], rinv[:])
                    nc.vector.tensor_mul(
                        ctxt[u][o:o + 64, j * 512:(j + 1) * 512],
                        avs[p][0:64, :],
                        bcast[:],
                    )

        # ---- out projection ----
        for rc in range(16):
            for n in range(2):
                ops = ps.tile([128, 512], f32, tag="a", bufs=3, name="ops")
                for u in range(2):
                    nc.tensor.matmul(
                        ops[:],
                        ctxt[u][:, rc * 128:(rc + 1) * 128],
                        wo_sb[:, u * DIN + n * 512: u * DIN + n * 512 + 512],
                        start=(u == 0),
                        stop=False,
                    )
                nc.tensor.matmul(
                    ops[:],
                    ones_row[:],
                    bo_sb[0:1, n * 512:(n + 1) * 512],
                    start=False,
                    stop=True,
                )
                osb = ob.tile([128, 512], f32, tag="o", name="osb")
                nc.vector.tensor_copy(osb[:], ops[:])
                nc.sync.dma_start(
                    out_d[rc * 128:(rc + 1) * 128, n * 512:(n + 1) * 512], osb[:]
                )

    nc.compile()
    return nc


_NC = None


def _get_nc():
    global _NC
    if _NC is None:
        _NC = build_nc()
    return _NC


def kernel(x, W_q, W_k, W_v, W_o, b_o):
    global LAST_RESULTS
    nc = _get_nc()
    bf = ml_dtypes.bfloat16
    x = np.ascontiguousarray(np.asarray(x, np.float32)).astype(bf)
    W_q = np.asarray(W_q, np.float32).astype(bf)
    W_k = np.asarray(W_k, np.float32).astype(bf)
    W_v = np.asarray(W_v, np.float32).astype(bf)
    W_o = np.asarray(W_o, np.float32).astype(bf)
    b_o = np.asarray(b_o, np.float32).astype(bf).reshape(1, DIN)
    zeros_bo = np.zeros((1, DIN), bf)

    in_maps = []
    for c in range(8):
        bi, g = c // 4, c % 4
        sl = slice(g * QC, (g + 1) * QC)
        in_maps.append({
            "x": x[bi],
            "wq": np.ascontiguousarray(W_q[:, sl]),
            "wk": np.ascontiguousarray(W_k[:, sl]),
            "wv": np.ascontiguousarray(W_v[:, sl]),
            "wo": np.ascontiguousarray(W_o[sl, :]),
            "bo": b_o if g == 0 else zeros_bo,
        })

    res = run_bass_kernel_spmd(nc, in_maps, list(range(8)), trace=TRACE)
    LAST_RESULTS = res
    outs = [r["out"] for r in res.results]
    return np.stack([
        outs[0] + outs[1] + outs[2] + outs[3],
        outs[4] + outs[5] + outs[6] + outs[7],
    ])


if __name__ == "__main__":
    if "--compile-only" in sys.argv:
        import tempfile
        from concourse.bass_utils import compile_bass_kernel

        nc = build_nc()
        with tempfile.TemporaryDirectory() as td:
            print("walrus compiling...")
            neff = compile_bass_kernel(nc, td)
            print("COMPILE OK", neff)


# revision 11
# speedup vs baseline: 1.5010x; 1.1391x over previous
"""Trainium2 Bass kernel for nn_BaseAttention (causal MHA, b=2, n=2048, d=1024, 16 heads).

Sharding (8 cores): core c handles batch c//4 and heads 4*(c%4)..4*(c%4)+3.
- W_q/W_k/W_v column-sharded (256 cols/core), W_o row-sharded (256 rows/core).
- Each core computes a partial output [2048, 1024] in fp32; host sums the 4
  partials per batch (row-parallel out-projection) and stacks the 2 batches.

Per-core kernel (bf16 data path, fp32 PSUM accumulation):
  - x (bf16 on host) is DMA-transposed straight into SBUF as x^T.
  - Q^T/K^T projections emitted transposed; V natural with a ones column per
    head ([V|1] trick: AV matmul yields ctx^T and the softmax row-sum at psum
    partition 64 in one pass).
  - attention per (head-pair, q-tile): S^T on PE (even/odd heads on disjoint
    PE row-halves -> concurrent), exp on ACT over [128,1024] psum pairs,
    causal mask via gpsimd affine_select, AV pipelined one i-pair behind,
    normalization via DVE reciprocal_approx_fast + gpsimd partition_broadcast.
  - projection work of round g+1 (or out-projection chunks in the last round)
    is woven between attention steps so the in-order PE queue never idles
    while ACT catches up (keeps HAM warm).
  - out-projection adds the bias via a rank-1 (ones x b_o) matmul into PSUM.
"""
import sys, types

sys.path.insert(0, "/opt/trn_rl_repo")


def _install_ntff_shim():
    # antenv.axon_hooks is absent in this image; register the NTFF profile
    # hook via ctypes so run_bass_kernel_spmd(trace=True) works under axon.
    if "antenv.axon_hooks" in sys.modules:
        return
    try:
        sys.path.insert(0, "/root/.axon_site")
        from trn_agent_boot.trn_boot import _ntff_profile_via_ctypes

        hook = _ntff_profile_via_ctypes("/opt/axon/libaxon_pjrt.so")
        mod = types.ModuleType("antenv.axon_hooks")
        mod.get_axon_ntff_profile_hook = lambda: hook
        mod.set_axon_ntff_profile_hook = lambda h: None
        sys.modules["antenv.axon_hooks"] = mod
    except Exception:
        pass


_install_ntff_shim()

import numpy as np
import ml_dtypes
import concourse.bass as bass
import concourse.mybir as mybir
import concourse.tile as tile
from concourse import bacc
from concourse.bass_utils import run_bass_kernel_spmd
from contextlib import ExitStack

f32 = mybir.dt.float32
bf16 = mybir.dt.bfloat16
EXP = mybir.ActivationFunctionType.Exp

SEQ = 2048          # sequence length
DIN = 1024          # model dim (8 chunks of 128)
QC = 256            # q/k/v cols per core (4 heads x 64)
HD = 64             # head dim
NH = 4              # heads per core
NG = 4              # row groups of 512
VST = NH * 65       # Vones stride per row chunk (4 heads x (64 V + 1 ones))

TRACE = False
LAST_RESULTS = None


def build_nc():
    nc = bacc.Bacc()
    x_d = nc.dram_tensor("x", [SEQ, DIN], bf16, kind="ExternalInput")
    wq_d = nc.dram_tensor("wq", [DIN, QC], bf16, kind="ExternalInput")
    wk_d = nc.dram_tensor("wk", [DIN, QC], bf16, kind="ExternalInput")
    wv_d = nc.dram_tensor("wv", [DIN, QC], bf16, kind="ExternalInput")
    wo_d = nc.dram_tensor("wo", [QC, DIN], bf16, kind="ExternalInput")
    bo_d = nc.dram_tensor("bo", [1, DIN], bf16, kind="ExternalInput")
    out_d = nc.dram_tensor("out", [SEQ, DIN], f32, kind="ExternalOutput")

    with tile.TileContext(nc) as tc, ExitStack() as ctx:
        cst = ctx.enter_context(tc.tile_pool(name="cst", bufs=1))
        wr = ctx.enter_context(tc.tile_pool(name="wr", bufs=1))
        big = ctx.enter_context(tc.tile_pool(name="big", bufs=1))
        ptp = ctx.enter_context(tc.tile_pool(name="ptp", bufs=6))
        nrm = ctx.enter_context(tc.tile_pool(name="nrm", bufs=2))
        ob = ctx.enter_context(tc.tile_pool(name="ob", bufs=3))
        ps = ctx.enter_context(tc.tile_pool(name="ps", bufs=1, space="PSUM"))

        # ---- constants ----
        ones_row = cst.tile([1, 128], bf16)
        nc.vector.memset(ones_row[:], 1.0)

        # ---- DMAs, ordered by first use ----
        def dma_w(wd, n_inner):
            name = wd.name + "_sb"
            t = wr.tile([128, 8 * n_inner], bf16, name=name)
            nc.sync.dma_start(
                t[:].rearrange("p (c n) -> p c n", n=n_inner),
                wd.rearrange("(c p) n -> p c n", p=128),
            )
            return t

        wq_sb = dma_w(wq_d, QC)
        wk_sb = dma_w(wk_d, QC)

        # x^T via DMA transpose: xT[p, c*2048+r] = x[r, c*128+p]; group 0 first
        xT = big.tile([128, 8 * SEQ], bf16)
        for g in range(NG):
            if g == 1:
                wv_sb = dma_w(wv_d, QC)
            for c in range(8):
                nc.sync.dma_start_transpose(
                    xT[:, c * SEQ + g * 512: c * SEQ + (g + 1) * 512],
                    x_d[g * 512:(g + 1) * 512, c * 128:(c + 1) * 128],
                )
        wo_sb = cst.tile([128, 2 * DIN], bf16)
        nc.sync.dma_start(
            wo_sb[:].rearrange("p (u n) -> p u n", n=DIN),
            wo_d.rearrange("(u p) n -> p u n", p=128),
        )
        bo_sb = cst.tile([1, DIN], bf16)
        nc.sync.dma_start(bo_sb[:], bo_d[:])

        # ---- persistent activations ----
        qt_sb = [big.tile([128, SEQ], bf16, name=f"qt{t}") for t in range(2)]
        kt_sb = [big.tile([128, SEQ], bf16, name=f"kt{t}") for t in range(2)]
        vones = big.tile([128, 16 * VST], bf16)
        ctxt = [big.tile([128, SEQ], bf16, name=f"ctxt{t}") for t in range(2)]

        vview = vones.rearrange("p (r h e) -> p r h e", h=NH, e=65)
        nc.vector.memset(vview[:, :, :, 64], 1.0)

        # ---- emission helpers ----
        def emit_qk(g, t, wt, dst):
            prj = ps.tile([128, 512], f32, tag="b", bufs=2, name="prj")
            for c in range(8):
                nc.tensor.matmul(
                    prj[:],
                    wt[:, c * QC + t * 128: c * QC + t * 128 + 128],
                    xT[:, c * SEQ + g * 512: c * SEQ + g * 512 + 512],
                    start=(c == 0),
                    stop=(c == 7),
                )
            nc.vector.tensor_copy(dst[t][:, g * 512:(g + 1) * 512], prj[:])

        def emit_v(g, rc):
            rcg = 4 * g + rc
            vps = ps.tile([128, 256], f32, tag="b", bufs=2, name="vps")
            for c in range(8):
                nc.tensor.matmul(
                    vps[:],
                    xT[:, c * SEQ + rcg * 128: c * SEQ + rcg * 128 + 128],
                    wv_sb[:, c * QC:(c + 1) * QC],
                    start=(c == 0),
                    stop=(c == 7),
                )
            nc.vector.tensor_copy(
                vview[:, rcg, :, 0:64],
                vps[:].rearrange("p (h e) -> p h e", e=HD),
            )

        def emit_outproj(rc, n):
            ops = ps.tile([128, 512], f32, tag="b", bufs=2, name="ops")
            for u in range(2):
                nc.tensor.matmul(
                    ops[:],
                    ctxt[u][:, rc * 128:(rc + 1) * 128],
                    wo_sb[:, u * DIN + n * 512: u * DIN + n * 512 + 512],
                    start=(u == 0),
                    stop=False,
                )
            nc.tensor.matmul(
                ops[:],
                ones_row[:],
                bo_sb[0:1, n * 512:(n + 1) * 512],
                start=False,
                stop=True,
            )
            osb = ob.tile([128, 512], f32, tag="o", name="osb")
            nc.vector.tensor_copy(osb[:], ops[:])
            nc.sync.dma_start(
                out_d[rc * 128:(rc + 1) * 128, n * 512:(n + 1) * 512], osb[:]
            )

        def proj_chunks(g):
            for t in range(2):
                yield lambda t=t: emit_qk(g, t, wq_sb, qt_sb)
                yield lambda t=t: emit_qk(g, t, wk_sb, kt_sb)
            for rc in range(4):
                yield lambda rc=rc: emit_v(g, rc)

        # ---- round 0 projections up-front ----
        for f in proj_chunks(0):
            f()

        # ---- main rounds: attention(j=g) woven with proj(g+1)/outproj ----
        for g in range(NG):
            j = g
            imax = 4 * j + 3
            npair = (imax + 1) // 2
            if g < NG - 1:
                filler = list(proj_chunks(g + 1))
            else:
                filler = [
                    (lambda rc=rc, n=n: emit_outproj(rc, n))
                    for rc in range(12)
                    for n in range(2)
                ]
            steps_total = 2 * npair
            fill_i = 0
            step = 0

            for u in range(2):           # head pair u: heads 2u, 2u+1
                avs = [ps.tile([65, 512], f32, tag="av", bufs=2, name=f"av{p}")
                       for p in range(2)]
                pts = [[], []]           # per parity: list of [128,1024] pair tiles
                for ip in range(npair):
                    i0 = 2 * ip
                    cur = []
                    for p in range(2):
                        sps = ps.tile([128, 1024], f32, tag="a", bufs=2, name="sps")
                        cur.append(sps)
                    for half in range(2):
                        i = i0 + half
                        for p in range(2):
                            o = p * 64
                            nc.tensor.matmul(
                                cur[p][:, half * 512:(half + 1) * 512],
                                kt_sb[u][o:o + 64, i * 128:(i + 1) * 128],
                                qt_sb[u][o:o + 64, j * 512:(j + 1) * 512],
                                start=True,
                                stop=True,
                            )
                    for p in range(2):
                        pt = ptp.tile([128, 1024], bf16, tag="pt", name="pt")
                        nc.scalar.activation(pt[:], cur[p][:], EXP, scale=0.125)
                        for half in range(2):
                            i = i0 + half
                            if i >= 4 * j:
                                nc.gpsimd.affine_select(
                                    out=pt[:, half * 512:(half + 1) * 512],
                                    in_=pt[:, half * 512:(half + 1) * 512],
                                    compare_op=mybir.AluOpType.is_ge,
                                    fill=0.0,
                                    base=512 * j - 128 * i,
                                    channel_multiplier=-1,
                                    pattern=[[1, 512]],
                                )
                        pts[p].append(pt)
                    if ip >= 1:
                        kp = ip - 1
                        for p in range(2):
                            h = 2 * u + p
                            for half in range(2):
                                k = 2 * kp + half
                                nc.tensor.matmul(
                                    avs[p][:],
                                    vones[:, k * VST + h * 65: k * VST + h * 65 + 65],
                                    pts[p][kp][:, half * 512:(half + 1) * 512],
                                    start=(k == 0),
                                    stop=False,
                                )
                    step += 1
                    want = (len(filler) * step) // steps_total
                    while fill_i < want:
                        filler[fill_i]()
                        fill_i += 1
                # tail AVs for the last pair
                kp = npair - 1
                for p in range(2):
                    h = 2 * u + p
                    for half in range(2):
                        k = 2 * kp + half
                        nc.tensor.matmul(
                            avs[p][:],
                            vones[:, k * VST + h * 65: k * VST + h * 65 + 65],
                            pts[p][kp][:, half * 512:(half + 1) * 512],
                            start=(k == 0),
                            stop=(half == 1),
                        )
                # normalize: ctx^T = av[0:64] * (1/rowsum) broadcast
                for p in range(2):
                    o = p * 64
                    rsrow = nrm.tile([1, 512], f32, tag="rsrow", name="rsrow")
                    nc.vector.tensor_copy(rsrow[:], avs[p][64:65, :])
                    rinv = nrm.tile([1, 512], f32, tag="rinv", name="rinv")
                    nc.vector.reciprocal_approx_fast(rinv[:], rsrow[:])
                    bcast = nrm.tile([64, 512], f32, tag="bcast", name="bcast")
                    nc.gpsimd.partition_broadcast(bcast[:], rinv[:])
                    nc.vector.tensor_mul(
                        ctxt[u][o:o + 64, j * 512:(j + 1) * 512],
                        avs[p][0:64, :],
                        bcast[:],
                    )
            while fill_i < len(filler):
                filler[fill_i]()
                fill_i += 1

        # ---- final out-projection chunks ----
        for rc in range(12, 16):
            for n in range(2):
                emit_outproj(rc, n)

    nc.compile()
    return nc


_NC = None


def _get_nc():
    global _NC
    if _NC is None:
        _NC = build_nc()
    return _NC


def kernel(x, W_q, W_k, W_v, W_o, b_o):
    global LAST_RESULTS
    nc = _get_nc()
    bf = ml_dtypes.bfloat16
    x = np.ascontiguousarray(np.asarray(x, np.float32)).astype(bf)
    W_q = np.asarray(W_q, np.float32).astype(bf)
    W_k = np.asarray(W_k, np.float32).astype(bf)
    W_v = np.asarray(W_v, np.float32).astype(bf)
    W_o = np.asarray(W_o, np.float32).astype(bf)
    b_o = np.asarray(b_o, np.float32).astype(bf).reshape(1, DIN)
    zeros_bo = np.zeros((1, DIN), bf)

    in_maps = []
    for c in range(8):
        bi, g = c // 4, c % 4
        sl = slice(g * QC, (g + 1) * QC)
        in_maps.append({
            "x": x[bi],
            "wq": np.ascontiguousarray(W_q[:, sl]),
            "wk": np.ascontiguousarray(W_k[:, sl]),
            "wv": np.ascontiguousarray(W_v[:, sl]),
            "wo": np.ascontiguousarray(W_o[sl, :]),
            "bo": b_o if g == 0 else zeros_bo,
        })

    res = run_bass_kernel_spmd(nc, in_maps, list(range(8)), trace=TRACE)
    LAST_RESULTS = res
    outs = [r["out"] for r in res.results]
    return np.stack([
        outs[0] + outs[1] + outs[2] + outs[3],
        outs[4] + outs[5] + outs[6] + outs[7],
    ])


if __name__ == "__main__":
    if "--compile-only" in sys.argv:
        import tempfile
        from concourse.bass_utils import compile_bass_kernel

        nc = build_nc()
        with tempfile.TemporaryDirectory() as td:
            print("walrus compiling...")
            neff = compile_bass_kernel(nc, td)
            print("COMPILE OK", neff)


# revision 12
# speedup vs baseline: 1.5033x; 1.0015x over previous
"""Trainium2 Bass kernel for nn_BaseAttention (causal MHA, b=2, n=2048, d=1024, 16 heads).

Sharding (8 cores): core c handles batch c//4 and heads 4*(c%4)..4*(c%4)+3.
- W_q/W_k/W_v column-sharded (256 cols/core), W_o row-sharded (256 rows/core).
- Each core computes a partial output [2048, 1024] in fp32; host sums the 4
  partials per batch (row-parallel out-projection) and stacks the 2 batches.

Per-core kernel (bf16 data path, fp32 PSUM accumulation):
  - x (bf16 on host) is DMA-transposed straight into SBUF as x^T.
  - Q^T/K^T projections emitted transposed; V natural with a ones column per
    head ([V|1] trick: AV matmul yields ctx^T and the softmax row-sum at psum
    partition 64 in one pass).
  - attention per (head-pair, q-tile): S^T on PE (even/odd heads on disjoint
    PE row-halves -> concurrent), exp on ACT over [128,1024] psum pairs,
    causal mask via gpsimd affine_select, AV pipelined one i-pair behind,
    normalization via DVE reciprocal_approx_fast + gpsimd partition_broadcast.
  - projection work of round g+1 (or out-projection chunks in the last round)
    is woven between attention steps so the in-order PE queue never idles
    while ACT catches up (keeps HAM warm).
  - out-projection adds the bias via a rank-1 (ones x b_o) matmul into PSUM.
"""
import sys, types

sys.path.insert(0, "/opt/trn_rl_repo")


def _install_ntff_shim():
    # antenv.axon_hooks is absent in this image; register the NTFF profile
    # hook via ctypes so run_bass_kernel_spmd(trace=True) works under axon.
    if "antenv.axon_hooks" in sys.modules:
        return
    try:
        sys.path.insert(0, "/root/.axon_site")
        from trn_agent_boot.trn_boot import _ntff_profile_via_ctypes

        hook = _ntff_profile_via_ctypes("/opt/axon/libaxon_pjrt.so")
        mod = types.ModuleType("antenv.axon_hooks")
        mod.get_axon_ntff_profile_hook = lambda: hook
        mod.set_axon_ntff_profile_hook = lambda h: None
        sys.modules["antenv.axon_hooks"] = mod
    except Exception:
        pass


_install_ntff_shim()

import numpy as np
import ml_dtypes
import concourse.bass as bass
import concourse.mybir as mybir
import concourse.tile as tile
from concourse import bacc
from concourse.bass_utils import run_bass_kernel_spmd
from contextlib import ExitStack

f32 = mybir.dt.float32
bf16 = mybir.dt.bfloat16
EXP = mybir.ActivationFunctionType.Exp

SEQ = 2048          # sequence length
DIN = 1024          # model dim (8 chunks of 128)
QC = 256            # q/k/v cols per core (4 heads x 64)
HD = 64             # head dim
NH = 4              # heads per core
NG = 4              # row groups of 512
VST = NH * 65       # Vones stride per row chunk (4 heads x (64 V + 1 ones))

TRACE = False
LAST_RESULTS = None


def build_nc():
    nc = bacc.Bacc()
    x_d = nc.dram_tensor("x", [SEQ, DIN], bf16, kind="ExternalInput")
    wq_d = nc.dram_tensor("wq", [DIN, QC], bf16, kind="ExternalInput")
    wk_d = nc.dram_tensor("wk", [DIN, QC], bf16, kind="ExternalInput")
    wv_d = nc.dram_tensor("wv", [DIN, QC], bf16, kind="ExternalInput")
    wo_d = nc.dram_tensor("wo", [QC, DIN], bf16, kind="ExternalInput")
    bo_d = nc.dram_tensor("bo", [1, DIN], bf16, kind="ExternalInput")
    out_d = nc.dram_tensor("out", [SEQ, DIN], f32, kind="ExternalOutput")

    with tile.TileContext(nc) as tc, ExitStack() as ctx:
        cst = ctx.enter_context(tc.tile_pool(name="cst", bufs=1))
        wr = ctx.enter_context(tc.tile_pool(name="wr", bufs=1))
        big = ctx.enter_context(tc.tile_pool(name="big", bufs=1))
        ptp = ctx.enter_context(tc.tile_pool(name="ptp", bufs=6))
        nrm = ctx.enter_context(tc.tile_pool(name="nrm", bufs=2))
        ob = ctx.enter_context(tc.tile_pool(name="ob", bufs=3))
        ps = ctx.enter_context(tc.tile_pool(name="ps", bufs=1, space="PSUM"))

        # ---- DMAs, ordered by first use ----
        def dma_w(wd, n_inner):
            name = wd.name + "_sb"
            t = wr.tile([128, 8 * n_inner], bf16, name=name)
            nc.sync.dma_start(
                t[:].rearrange("p (c n) -> p c n", n=n_inner),
                wd.rearrange("(c p) n -> p c n", p=128),
            )
            return t

        wq_sb = dma_w(wq_d, QC)

        # x^T via DMA transpose: xT[p, c*2048+r] = x[r, c*128+p]; group 0 first
        xT = big.tile([128, 8 * SEQ], bf16)
        for g in range(NG):
            if g == 1:
                wk_sb = dma_w(wk_d, QC)
                wv_sb = dma_w(wv_d, QC)
            for c in range(8):
                nc.sync.dma_start_transpose(
                    xT[:, c * SEQ + g * 512: c * SEQ + (g + 1) * 512],
                    x_d[g * 512:(g + 1) * 512, c * 128:(c + 1) * 128],
                )
        wo_sb = cst.tile([128, 2 * DIN], bf16)
        nc.sync.dma_start(
            wo_sb[:].rearrange("p (u n) -> p u n", n=DIN),
            wo_d.rearrange("(u p) n -> p u n", p=128),
        )
        bo_sb = cst.tile([1, DIN], bf16)
        nc.sync.dma_start(bo_sb[:], bo_d[:])
        bo_f = cst.tile([1, DIN], f32)
        nc.vector.tensor_copy(bo_f[:], bo_sb[:])
        bias_bc = cst.tile([128, DIN], f32)
        nc.gpsimd.partition_broadcast(bias_bc[:], bo_f[:])

        # ---- persistent activations ----
        qt_sb = [big.tile([128, SEQ], bf16, name=f"qt{t}") for t in range(2)]
        kt_sb = [big.tile([128, SEQ], bf16, name=f"kt{t}") for t in range(2)]
        vones = big.tile([128, 16 * VST], bf16)
        ctxt = [big.tile([128, SEQ], bf16, name=f"ctxt{t}") for t in range(2)]

        vview = vones.rearrange("p (r h e) -> p r h e", h=NH, e=65)
        nc.vector.memset(vview[:, :, :, 64], 1.0)

        # ---- emission helpers ----
        def emit_qk(g, t, wt, dst):
            prj = ps.tile([128, 512], f32, tag="b", bufs=2, name="prj")
            for c in range(8):
                nc.tensor.matmul(
                    prj[:],
                    wt[:, c * QC + t * 128: c * QC + t * 128 + 128],
                    xT[:, c * SEQ + g * 512: c * SEQ + g * 512 + 512],
                    start=(c == 0),
                    stop=(c == 7),
                )
            nc.vector.tensor_copy(dst[t][:, g * 512:(g + 1) * 512], prj[:])

        def emit_v(g, rc):
            rcg = 4 * g + rc
            vps = ps.tile([128, 256], f32, tag="b", bufs=2, name="vps")
            for c in range(8):
                nc.tensor.matmul(
                    vps[:],
                    xT[:, c * SEQ + rcg * 128: c * SEQ + rcg * 128 + 128],
                    wv_sb[:, c * QC:(c + 1) * QC],
                    start=(c == 0),
                    stop=(c == 7),
                )
            nc.vector.tensor_copy(
                vview[:, rcg, :, 0:64],
                vps[:].rearrange("p (h e) -> p h e", e=HD),
            )

        def emit_outproj(rc, n):
            ops = ps.tile([128, 512], f32, tag="b", bufs=2, name="ops")
            for u in range(2):
                nc.tensor.matmul(
                    ops[:],
                    ctxt[u][:, rc * 128:(rc + 1) * 128],
                    wo_sb[:, u * DIN + n * 512: u * DIN + n * 512 + 512],
                    start=(u == 0),
                    stop=(u == 1),
                )
            osb = ob.tile([128, 512], f32, tag="o", name="osb")
            nc.vector.tensor_add(osb[:], ops[:], bias_bc[:, n * 512:(n + 1) * 512])
            nc.sync.dma_start(
                out_d[rc * 128:(rc + 1) * 128, n * 512:(n + 1) * 512], osb[:]
            )

        def proj_chunks(g):
            for t in range(2):
                yield lambda t=t: emit_qk(g, t, wq_sb, qt_sb)
                yield lambda t=t: emit_qk(g, t, wk_sb, kt_sb)
            for rc in range(4):
                yield lambda rc=rc: emit_v(g, rc)

        # ---- round 0 projections up-front ----
        for f in proj_chunks(0):
            f()

        # ---- main rounds: attention(j=g) woven with proj(g+1)/outproj ----
        for g in range(NG):
            j = g
            imax = 4 * j + 3
            npair = (imax + 1) // 2
            if g < NG - 1:
                filler = list(proj_chunks(g + 1))
            else:
                filler = [
                    (lambda rc=rc, n=n: emit_outproj(rc, n))
                    for rc in range(12)
                    for n in range(2)
                ]
            steps_total = 2 * npair
            fill_i = 0
            step = 0

            for u in range(2):           # head pair u: heads 2u, 2u+1
                avs = [ps.tile([65, 512], f32, tag="av", bufs=2, name=f"av{p}")
                       for p in range(2)]
                pts = [[], []]           # per parity: list of [128,1024] pair tiles
                for ip in range(npair):
                    i0 = 2 * ip
                    cur = []
                    for p in range(2):
                        sps = ps.tile([128, 1024], f32, tag="a", bufs=2, name="sps")
                        cur.append(sps)
                    for half in range(2):
                        i = i0 + half
                        off = max(0, 128 * i - 512 * j)
                        for p in range(2):
                            o = p * 64
                            nc.tensor.matmul(
                                cur[p][:, half * 512 + off:(half + 1) * 512],
                                kt_sb[u][o:o + 64, i * 128:(i + 1) * 128],
                                qt_sb[u][o:o + 64, j * 512 + off:(j + 1) * 512],
                                start=True,
                                stop=True,
                            )
                    for p in range(2):
                        pt = ptp.tile([128, 1024], bf16, tag="pt", name="pt")
                        nc.scalar.activation(pt[:], cur[p][:], EXP, scale=0.125)
                        for half in range(2):
                            i = i0 + half
                            if i >= 4 * j:
                                off = 128 * i - 512 * j
                                nc.gpsimd.affine_select(
                                    out=pt[:, half * 512 + off:(half + 1) * 512],
                                    in_=pt[:, half * 512 + off:(half + 1) * 512],
                                    compare_op=mybir.AluOpType.is_ge,
                                    fill=0.0,
                                    base=0,
                                    channel_multiplier=-1,
                                    pattern=[[1, 512 - off]],
                                )
                        pts[p].append(pt)
                    if ip >= 1:
                        kp = ip - 1
                        for p in range(2):
                            h = 2 * u + p
                            for half in range(2):
                                k = 2 * kp + half
                                off = max(0, 128 * k - 512 * j)
                                nc.tensor.matmul(
                                    avs[p][:, off:512],
                                    vones[:, k * VST + h * 65: k * VST + h * 65 + 65],
                                    pts[p][kp][:, half * 512 + off:(half + 1) * 512],
                                    start=(k == 0),
                                    stop=False,
                                )
                    step += 1
                    want = (len(filler) * step) // steps_total
                    while fill_i < want:
                        filler[fill_i]()
                        fill_i += 1
                # tail AVs for the last pair
                kp = npair - 1
                for p in range(2):
                    h = 2 * u + p
                    for half in range(2):
                        k = 2 * kp + half
                        off = max(0, 128 * k - 512 * j)
                        nc.tensor.matmul(
                            avs[p][:, off:512],
                            vones[:, k * VST + h * 65: k * VST + h * 65 + 65],
                            pts[p][kp][:, half * 512 + off:(half + 1) * 512],
                            start=(k == 0),
                            stop=(half == 1),
                        )
                # normalize: ctx^T = av[0:64] * (1/rowsum) broadcast
                for p in range(2):
                    o = p * 64
                    rsrow = nrm.tile([1, 512], f32, tag="rsrow", name="rsrow")
                    nc.vector.tensor_copy(rsrow[:], avs[p][64:65, :])
                    rinv = nrm.tile([1, 512], f32, tag="rinv", name="rinv")
                    nc.vector.reciprocal_approx_fast(rinv[:], rsrow[:])
                    bcast = nrm.tile([64, 512], f32, tag="bcast", name="bcast")
                    nc.gpsimd.partition_broadcast(bcast[:], rinv[:])
                    nc.vector.tensor_mul(
                        ctxt[u][o:o + 64, j * 512:(j + 1) * 512],
                        avs[p][0:64, :],
                        bcast[:],
                    )
            while fill_i < len(filler):
                filler[fill_i]()
                fill_i += 1

        # ---- final out-projection chunks ----
        for rc in range(12, 16):
            for n in range(2):
                emit_outproj(rc, n)

    nc.compile()
    return nc


_NC = None


def _get_nc():
    global _NC
    if _NC is None:
        _NC = build_nc()
    return _NC


def kernel(x, W_q, W_k, W_v, W_o, b_o):
    global LAST_RESULTS
    nc = _get_nc()
    bf = ml_dtypes.bfloat16
    x = np.ascontiguousarray(np.asarray(x, np.float32)).astype(bf)
    W_q = np.asarray(W_q, np.float32).astype(bf)
    W_k = np.asarray(W_k, np.float32).astype(bf)
    W_v = np.asarray(W_v, np.float32).astype(bf)
    W_o = np.asarray(W_o, np.float32).astype(bf)
    b_o = np.asarray(b_o, np.float32).astype(bf).reshape(1, DIN)
    zeros_bo = np.zeros((1, DIN), bf)

    in_maps = []
    for c in range(8):
        bi, g = c // 4, c % 4
        sl = slice(g * QC, (g + 1) * QC)
        in_maps.append({
            "x": x[bi],
            "wq": np.ascontiguousarray(W_q[:, sl]),
            "wk": np.ascontiguousarray(W_k[:, sl]),
            "wv": np.ascontiguousarray(W_v[:, sl]),
            "wo": np.ascontiguousarray(W_o[sl, :]),
            "bo": b_o if g == 0 else zeros_bo,
        })

    res = run_bass_kernel_spmd(nc, in_maps, list(range(8)), trace=TRACE)
    LAST_RESULTS = res
    outs = [r["out"] for r in res.results]
    return np.stack([
        outs[0] + outs[1] + outs[2] + outs[3],
        outs[4] + outs[5] + outs[6] + outs[7],
    ])


if __name__ == "__main__":
    if "--compile-only" in sys.argv:
        import tempfile
        from concourse.bass_utils import compile_bass_kernel

        nc = build_nc()
        with tempfile.TemporaryDirectory() as td:
            print("walrus compiling...")
            neff = compile_bass_kernel(nc, td)
            print("COMPILE OK", neff)
